# revision 15
# baseline (speedup 1.0000x reference)
"""DinoV2 backbone wrapper — 8-core Trainium2 Bass kernel.

Sharding: core c -> view v=c//4, head-group g=c%4 (heads 3g..3g+2, mlp hidden
chunk g). Feature-major activations [C=768 partitions x T tokens free].
fp32r matmuls (tokens padded 577->640, all matmul free dims even).
Per layer: AllGather(4-group) of attention head outputs -> full proj on every
core; AllReduce(4-group) of fc2 partials. Global layers additionally pair-
AllGather roped k + v between the two view-cores.

Host-side folds: LN1/LN2 scale+bias into qkv/v/fc1 weights+biases; layerscale
gamma into proj/fc2 weight columns; RoPE rotation into extra weight columns
(q_rot, k_rot); v bias applied post-attention (softmax sums to 1).
"""
import sys
import numpy as np

sys.path.insert(0, '/root/.axon_site')

import concourse.bass as bass
import concourse.bacc as bacc
import concourse.mybir as mybir
from concourse import tile
from concourse.bass_utils import run_bass_kernel_spmd
from contextlib import ExitStack

F32 = mybir.dt.float32
F32R = mybir.dt.float32r
AF = mybir.ActivationFunctionType
ALU = mybir.AluOpType

L, C, H, DH, S, N, B = 12, 768, 12, 64, 2, 577, 1
EPS = 1e-6
T = 577
TP = 640            # padded tokens (5*128)
KC = 5              # local key chunks
KCG = 10            # global key chunks
NCH = [(0, 512), (512, 128)]
NCORES = 8
MM_DT = F32R        # big-matmul dtype


def _rot_perm_sign():
    idx = np.zeros(DH, np.int64)
    sgn = np.zeros(DH, np.float32)
    for base in (0, 32):
        for d in range(16):
            idx[base + d] = base + d + 16
            sgn[base + d] = -1.0
            idx[base + 16 + d] = base + d
            sgn[base + 16 + d] = 1.0
    return idx, sgn


def _prep_core_inputs(c, inputs):
    v, g = c // 4, c % 4
    heads = [3 * g, 3 * g + 1, 3 * g + 2]
    x = np.asarray(inputs['x'])
    cam = np.asarray(inputs['camera_token'])
    qkv_w = np.asarray(inputs['qkv_w']); qkv_b = np.asarray(inputs['qkv_b'])
    qn = np.asarray(inputs['q_norm_w']); kn = np.asarray(inputs['k_norm_w'])
    pw = np.asarray(inputs['proj_w']); pb = np.asarray(inputs['proj_b'])
    g1 = np.asarray(inputs['ls1_gamma']); g2 = np.asarray(inputs['ls2_gamma'])
    n1w = np.asarray(inputs['norm1_w']); n1b = np.asarray(inputs['norm1_b'])
    n2w = np.asarray(inputs['norm2_w']); n2b = np.asarray(inputs['norm2_b'])
    f1w = np.asarray(inputs['fc1_w']); f1b = np.asarray(inputs['fc1_b'])
    f2w = np.asarray(inputs['fc2_w']); f2b = np.asarray(inputs['fc2_b'])
    fnw = np.asarray(inputs['final_norm_w']); fnb = np.asarray(inputs['final_norm_b'])
    cosl = np.asarray(inputs['rope_cos_local']); sinl = np.asarray(inputs['rope_sin_local'])
    cosg = np.asarray(inputs['rope_cos_global']); sing = np.asarray(inputs['rope_sin_global'])
    kvl = np.asarray(inputs['key_valid_local']); kvg = np.asarray(inputs['key_valid_global'])

    ridx, rsgn = _rot_perm_sign()

    xv = np.array(x[0, v])
    xv[0] = cam[0, v]
    x0 = np.zeros((C, TP), np.float32)
    x0[:, :T] = xv.T

    wqkv = np.zeros((L, C, 768), np.float32)
    wv = np.zeros((L, C, 198), np.float32)
    wproj = np.zeros((L, C, C), np.float32)
    wfc1 = np.zeros((L, C, C), np.float32)
    wfc2 = np.zeros((L, C, C), np.float32)
    vecs = np.zeros((L, 6, 6, 128), np.float32)
    bvv = np.zeros((L, 3, DH), np.float32)
    qb12 = np.zeros((L, 12, DH), np.float32)
    qnv = np.zeros((L, DH), np.float32); knv = np.zeros((L, DH), np.float32)

    def chan(vec):
        return vec.reshape(6, 128)

    for l in range(L):
        hrows = np.concatenate([np.arange(h * DH, (h + 1) * DH) for h in heads])
        q_w = qkv_w[l][hrows, :]; k_w = qkv_w[l][C + hrows, :]; v_w = qkv_w[l][2 * C + hrows, :]
        q_b = qkv_b[l][hrows]; k_b = qkv_b[l][C + hrows]; v_b = qkv_b[l][2 * C + hrows]

        def rot(mat, bias):
            m2 = np.zeros_like(mat); b2 = np.zeros_like(bias)
            for hh in range(3):
                blk = mat[hh * DH:(hh + 1) * DH]; bb = bias[hh * DH:(hh + 1) * DH]
                m2[hh * DH:(hh + 1) * DH] = rsgn[:, None] * blk[ridx]
                b2[hh * DH:(hh + 1) * DH] = rsgn * bb[ridx]
            return m2, b2
        qr_w, qr_b = rot(q_w, q_b)
        kr_w, kr_b = rot(k_w, k_b)
        big = np.concatenate([q_w, k_w, qr_w, kr_w], 0)
        bigb = np.concatenate([q_b, k_b, qr_b, kr_b], 0)
        wqkv[l] = (big * n1w[l][None, :]).T
        bqkv = bigb + big @ n1b[l]
        # v: token-major matmul, no bias in matmul (applied post-attention)
        vv = np.zeros((198, C), np.float32)
        for hh in range(3):
            vv[hh * 66:hh * 66 + DH] = v_w[hh * DH:(hh + 1) * DH]
        wv[l] = (vv * n1w[l][None, :]).T
        vbe = v_b + v_w @ n1b[l]          # effective per-channel v bias [192]
        bvv[l] = vbe.reshape(3, DH)
        wproj[l] = pw[l].T * g1[l][None, :]        # fold ls1 into proj columns
        f1 = f1w[l][768 * g:768 * (g + 1)]
        wfc1[l] = (f1 * n2w[l][None, :]).T
        bfc1 = f1b[l][768 * g:768 * (g + 1)] + f1 @ n2b[l]
        wfc2[l] = f2w[l][:, 768 * g:768 * (g + 1)].T * g2[l][None, :]  # fold ls2
        vecs[l, 0] = chan(g1[l] * pb[l])
        vecs[l, 1] = chan(g2[l] * f2b[l])
        vecs[l, 4] = chan(bqkv)
        vecs[l, 5] = chan(bfc1)
        qb12[l] = bqkv.reshape(12, 64)
        qnv[l] = qn[l]; knv[l] = kn[l]

    fvec = np.stack([chan(fnw), chan(fnb)], 0)

    def tabT(tab):
        out = np.zeros((DH, TP), np.float32); out[:, :T] = tab.T; return out
    cosl_t = tabT(cosl); sinl_t = tabT(sinl)
    cosg_t = tabT(cosg[v * T:(v + 1) * T]); sing_t = tabT(sing[v * T:(v + 1) * T])

    def maskify(kv, nkc):
        m = np.full(nkc * 128, -10000.0, np.float32)
        m[:kv.shape[0]] = (1.0 - kv) * -10000.0
        return m.reshape(nkc, 128)
    ml = maskify(kvl[v], KC)
    mg = np.concatenate([maskify(kvg[0, :T], KC), maskify(kvg[0, T:], KC)], 0)

    return {
        "x0": x0, "wqkv": wqkv, "wv": wv, "wproj": wproj, "wfc1": wfc1,
        "wfc2": wfc2, "vecs": vecs, "fvec": fvec, "bvv": bvv,
        "qnv": qnv, "knv": knv, "qb12": qb12,
        "cosl": cosl_t, "sinl": sinl_t, "cosg": cosg_t, "sing": sing_t,
        "ml": ml, "mg": mg,
    }


def build_nc(n_layers=L):
    nc = bacc.Bacc("TRN2", target_bir_lowering=False, debug=False, num_devices=NCORES)
    NL = n_layers
    d_x0 = nc.dram_tensor("x0", [C, TP], F32, kind="ExternalInput")
    d_wqkv = nc.dram_tensor("wqkv", [L, C, 768], MM_DT, kind="ExternalInput")
    d_wv = nc.dram_tensor("wv", [L, C, 198], MM_DT, kind="ExternalInput")
    d_wproj = nc.dram_tensor("wproj", [L, C, C], MM_DT, kind="ExternalInput")
    d_wfc1 = nc.dram_tensor("wfc1", [L, C, C], MM_DT, kind="ExternalInput")
    d_wfc2 = nc.dram_tensor("wfc2", [L, C, C], MM_DT, kind="ExternalInput")
    d_vecs = nc.dram_tensor("vecs", [L, 6, 6, 128], F32, kind="ExternalInput")
    d_fvec = nc.dram_tensor("fvec", [2, 6, 128], F32, kind="ExternalInput")
    d_bvv = nc.dram_tensor("bvv", [L, 3, DH], F32, kind="ExternalInput")
    d_qnv = nc.dram_tensor("qnv", [L, DH], F32, kind="ExternalInput")
    d_qb12 = nc.dram_tensor("qb12", [L, 12, DH], F32, kind="ExternalInput")
    d_knv = nc.dram_tensor("knv", [L, DH], F32, kind="ExternalInput")
    d_cosl = nc.dram_tensor("cosl", [DH, TP], F32, kind="ExternalInput")
    d_sinl = nc.dram_tensor("sinl", [DH, TP], F32, kind="ExternalInput")
    d_cosg = nc.dram_tensor("cosg", [DH, TP], F32, kind="ExternalInput")
    d_sing = nc.dram_tensor("sing", [DH, TP], F32, kind="ExternalInput")
    d_ml = nc.dram_tensor("ml", [KC, 128], F32, kind="ExternalInput")
    d_mg = nc.dram_tensor("mg", [KCG, 128], F32, kind="ExternalInput")
    d_out = nc.dram_tensor("outbuf", [5, 2, C, TP], F32, kind="ExternalOutput")

    G4 = [[0, 1, 2, 3], [4, 5, 6, 7]]
    G2 = [[0, 4], [1, 5], [2, 6], [3, 7]]
    KSZ = DH * 3 * TP
    VSZ = 128 * KC * 198
    KV_ELE = KSZ + VSZ

    out_slot = {2: 0, 5: 1, 8: 2, 11: 3}
    snap_slot = {2: 0, 4: 1, 8: 2, 10: 3, 11: 4}

    with tile.TileContext(nc) as tc, ExitStack() as ctx:
        sb = ctx.enter_context(tc.tile_pool(name="sb", bufs=1))
        wp = ctx.enter_context(tc.tile_pool(name="wp", bufs=2))
        psA = ctx.enter_context(tc.tile_pool(name="psA", bufs=2, space="PSUM"))
        psB = ctx.enter_context(tc.tile_pool(name="psB", bufs=2, space="PSUM"))
        dr = ctx.enter_context(tc.tile_pool(name="dr", bufs=2, space="DRAM"))

        x = sb.tile([128, 6, TP], F32, tag="x")
        nc.vector.memset(x[:], 0.0)
        nc.sync.dma_start(x[:], d_x0.ap().rearrange("(k p) t -> p k t", p=128))

        masks = sb.tile([128, KC + KCG], F32, tag="masks")
        nc.sync.dma_start(masks[:, 0:KC], d_ml.ap().rearrange("k p -> p k"))
        nc.sync.dma_start(masks[:, KC:], d_mg.ap().rearrange("k p -> p k"))
        ones1 = sb.tile([128, 2], F32, tag="ones1")
        nc.vector.memset(ones1[:], 1.0)
        eps1 = sb.tile([1, 1], F32, tag="eps1")
        nc.vector.memset(eps1[:], EPS)
        ones5 = sb.tile([128, KC, 1], F32, tag="ones5")
        nc.vector.memset(ones5[:], 1.0)

        def ln_rows(src6):
            p1 = psB.tile([1, TP], F32, tag="pav")
            p2 = psB.tile([1, TP], F32, tag="pav")
            for k in range(6):
                sq = sb.tile([128, TP], F32, tag="lnsq")
                nc.scalar.activation(out=sq[:], in_=src6[:, k, :], func=AF.Square)
                for (n0, nw) in NCH:
                    nc.tensor.matmul(p1[:, n0:n0 + nw], ones1[:, 0:1], src6[:, k, n0:n0 + nw],
                                     start=(k == 0), stop=(k == 5))
                    nc.tensor.matmul(p2[:, n0:n0 + nw], ones1[:, 0:1], sq[:, n0:n0 + nw],
                                     start=(k == 0), stop=(k == 5))
            mu = sb.tile([1, TP], F32, tag="rows", bufs=4)
            nc.scalar.mul(mu[:], p1[:], 1.0 / C)
            var = sb.tile([1, TP], F32, tag="rows", bufs=4)
            nc.vector.tensor_mul(var[:], mu[:], mu[:])
            nc.vector.scalar_tensor_tensor(out=var[:], in0=p2[:], scalar=1.0 / C,
                                           in1=var[:], op0=ALU.mult, op1=ALU.subtract)
            a = sb.tile([1, TP], F32, tag="rows", bufs=4)
            nc.scalar.activation(out=a[:], in_=var[:], func=AF.Sqrt, bias=eps1[:])
            nc.vector.reciprocal(a[:], a[:])
            cr = sb.tile([1, TP], F32, tag="rows", bufs=4)
            nc.vector.tensor_mul(cr[:], mu[:], a[:])
            nc.scalar.mul(cr[:], cr[:], -1.0)
            return a, cr

        def bcast(row, parts, tag):
            b = sb.tile([parts, TP], F32, tag=tag)
            nc.gpsimd.partition_broadcast(b[:], row[:])
            return b

        def ln_apply(src, a_b, c_b, out_dt, wrow=None, brow=None, tag="xn"):
            o = sb.tile([128, 6, TP], out_dt, tag=tag)
            for k in range(6):
                if wrow is None:
                    nc.vector.tensor_mul(o[:, k, :], src[:, k, :], a_b[:])
                    nc.gpsimd.tensor_add(o[:, k, :], o[:, k, :], c_b[:])
                else:
                    t1 = sb.tile([128, TP], F32, tag="lntmp", bufs=2)
                    nc.vector.tensor_mul(t1[:], src[:, k, :], a_b[:])
                    nc.vector.tensor_add(t1[:], t1[:], c_b[:])
                    nc.vector.tensor_scalar(out=o[:, k, :], in0=t1[:],
                                            scalar1=wrow[:, k:k + 1], scalar2=brow[:, k:k + 1],
                                            op0=ALU.mult, op1=ALU.add)
            return o

        for l in range(NL):
            is_global = (l % 2 == 1)
            nkc = KCG if is_global else KC
            moff = KC if is_global else 0
            tabs = sb.tile([DH, 2, TP], F32, tag="tabs", bufs=2)
            nc.sync.dma_start(tabs[:, 0, :], (d_cosg if is_global else d_cosl).ap())
            nc.sync.dma_start(tabs[:, 1, :], (d_sing if is_global else d_sinl).ap())
            ctab = tabs[:, 0, :]
            stab = tabs[:, 1, :]

            def wchunk(dram, m, tag):
                wt = wp.tile([128, 6, 128], MM_DT, tag=tag, bufs=2)
                nc.sync.dma_start(wt[:], dram.ap()[l].rearrange(
                    "(k p) m -> p k m", p=128)[:, :, m * 128:(m + 1) * 128])
                return wt
            wvt = wp.tile([128, 6, 198], MM_DT, tag="wvt")
            nc.sync.dma_start(wvt[:], d_wv.ap()[l].rearrange("(k p) m -> p k m", p=128))
            vec = wp.tile([128, 6, 6], F32, tag="vec")
            nc.sync.dma_start(vec[:], d_vecs.ap()[l].rearrange("v k p -> p v k"))
            vbt = wp.tile([DH, 3], F32, tag="vbt")
            nc.sync.dma_start(vbt[:], d_bvv.ap()[l].rearrange("h d -> d h"))
            qb = wp.tile([DH, 12], F32, tag="qb")
            nc.sync.dma_start(qb[:], d_qb12.ap()[l].rearrange("j d -> d j"))
            qkn = wp.tile([DH, 2], F32, tag="qkn")
            nc.sync.dma_start(qkn[:, 0:1], d_qnv.ap()[l].rearrange("(d o) -> d o", o=1))
            nc.sync.dma_start(qkn[:, 1:2], d_knv.ap()[l].rearrange("(d o) -> d o", o=1))

            # ---- LN1 + qkv
            a1, c1 = ln_rows(x)
            a1b = bcast(a1, 128, "a1b"); c1b = bcast(c1, 128, "c1b")
            xn = ln_apply(x, a1b, c1b, MM_DT, tag="xn")
            q_all = sb.tile([DH, 3, TP], F32, tag="q_all")
            k_all = sb.tile([DH, 3, TP], F32, tag="k_all")
            qr_all = sb.tile([DH, 3, TP], F32, tag="qr_all")
            kr_all = sb.tile([DH, 3, TP], F32, tag="kr_all")
            splits = (q_all, k_all, qr_all, kr_all)
            for m in range(6):
                wq = wchunk(d_wqkv, m, "wq")
                pq = psA.tile([128, TP], F32, tag="pbig")
                for k in range(6):
                    for (n0, nw) in NCH:
                        nc.tensor.matmul(pq[:, n0:n0 + nw], wq[:, k, :],
                                         xn[:, k, n0:n0 + nw], start=(k == 0), stop=(k == 5))
                for half in range(2):
                    j = 2 * m + half
                    dst = splits[j // 3]
                    nc.vector.tensor_scalar(out=dst[:, j % 3, :],
                                            in0=pq[half * DH:(half + 1) * DH, :],
                                            scalar1=qb[:, j:j + 1], scalar2=None, op0=ALU.add)
            # v token-major
            vsb = sb.tile([128, KC, 198], MM_DT, tag="vg")
            for tcn in range(KC):
                pv = psA.tile([128, 198], F32, tag="pbig")
                for k in range(6):
                    nc.tensor.matmul(pv[:], xn[:, k, tcn * 128:(tcn + 1) * 128],
                                     wvt[:, k, :], start=(k == 0), stop=(k == 5))
                nc.scalar.activation(out=vsb[:, tcn, :], in_=pv[:], func=AF.Copy)
            for hh in range(3):
                nc.vector.tensor_copy(out=vsb[:, :, hh * 66 + DH:hh * 66 + DH + 1],
                                      in_=ones5[:])

            qf = sb.tile([DH, 3, TP], MM_DT, tag="qf")
            kf = sb.tile([DH, 3, TP], MM_DT, tag="kg")
            for (dst, raw, rot_, nslot) in ((kf, k_all, kr_all, 1), (qf, q_all, qr_all, 0)):
                if dst is qf and is_global:
                    kvin = dr.tile([KV_ELE], MM_DT, tag="kvin")
                    kvout = dr.tile([2, KV_ELE], MM_DT, tag="kvout")
                    nc.sync.dma_start(kvin[0:KSZ].rearrange("(d h t) -> d h t", d=DH, h=3), kf[:])
                    nc.sync.dma_start(kvin[KSZ:].rearrange("(p a c) -> p a c", p=128, a=KC), vsb[:])
                    nc.gpsimd.collective_compute("AllGather", ALU.bypass, replica_groups=G2,
                                                 ins=[kvin.opt()], outs=[kvout.opt()])
                for hh in range(3):
                    src = raw[:, hh, :]
                    rsq = sb.tile([DH, TP], F32, tag="rsq")
                    nc.scalar.activation(out=rsq[:], in_=src, func=AF.Square)
                    pr = psB.tile([1, TP], F32, tag="pav")
                    for (n0, nw) in NCH:
                        nc.tensor.matmul(pr[:, n0:n0 + nw], ones1[0:DH, 0:1],
                                         rsq[:, n0:n0 + nw], start=True, stop=True)
                    rr = sb.tile([1, TP], F32, tag="rows", bufs=4)
                    nc.scalar.activation(out=rr[:], in_=pr[:], func=AF.Sqrt,
                                         bias=eps1[:], scale=1.0 / DH)
                    nc.vector.reciprocal(rr[:], rr[:])
                    rb = sb.tile([DH, TP], F32, tag="rb")
                    nc.gpsimd.partition_broadcast(rb[:], rr[:])
                    t1 = sb.tile([DH, TP], F32, tag="ropet1")
                    t2 = sb.tile([DH, TP], F32, tag="ropet2")
                    nc.vector.tensor_mul(t1[:], src, ctab)
                    nc.vector.tensor_mul(t2[:], rot_[:, hh, :], stab)
                    nc.vector.tensor_add(t1[:], t1[:], t2[:])
                    nc.vector.tensor_mul(t1[:], t1[:], rb[:])
                    nc.vector.tensor_scalar(out=dst[:, hh, :], in0=t1[:],
                                            scalar1=qkn[:, nslot:nslot + 1], scalar2=None,
                                            op0=ALU.mult)

            if is_global:
                kg = sb.tile([DH, 3, 2 * TP], MM_DT, tag="kg")
                vg = sb.tile([128, KCG, 198], MM_DT, tag="vg")
                for vv in range(2):
                    nc.sync.dma_start(
                        kg[:, :, vv * TP:(vv + 1) * TP],
                        kvout[vv, 0:KSZ].rearrange("(d h t) -> d h t", d=DH, h=3))
                    nc.sync.dma_start(
                        vg[:, vv * KC:(vv + 1) * KC, :],
                        kvout[vv, KSZ:].rearrange("(p a c) -> p a c", p=128, a=KC))
                k_use, v_use = kg, vg
            else:
                k_use, v_use = kf, vsb

            # ---- attention
            oown = sb.tile([DH, 3, TP], MM_DT, tag="oown")
            for hh in range(3):
                pav = psB.tile([66, TP], F32, tag="pav")
                for kt in range(nkc):
                    sc = psA.tile([128, TP], F32, tag="pbig")
                    for (n0, nw) in NCH:
                        nc.tensor.matmul(sc[:, n0:n0 + nw],
                                         k_use[:, hh, kt * 128:(kt + 1) * 128],
                                         qf[:, hh, n0:n0 + nw], start=True, stop=True)
                    ex = sb.tile([128, TP], MM_DT, tag="ex", bufs=2)
                    nc.scalar.activation(out=ex[:], in_=sc[:], func=AF.Exp,
                                         bias=masks[:, moff + kt:moff + kt + 1], scale=0.125)
                    for (n0, nw) in NCH:
                        nc.tensor.matmul(pav[:, n0:n0 + nw],
                                         v_use[:, kt, hh * 66:hh * 66 + 66],
                                         ex[:, n0:n0 + nw],
                                         start=(kt == 0), stop=(kt == nkc - 1))
                den = sb.tile([1, TP], F32, tag="rows", bufs=4)
                nc.vector.reciprocal(den[:], pav[DH:DH + 1, :])
                dnb = sb.tile([DH, TP], F32, tag="dnb")
                nc.gpsimd.partition_broadcast(dnb[:], den[:])
                nc.vector.tensor_mul(oown[:, hh, :], pav[0:DH, :], dnb[:])
                nc.vector.tensor_scalar(out=oown[:, hh, :], in0=oown[:, hh, :],
                                        scalar1=vbt[:, hh:hh + 1], scalar2=None, op0=ALU.add)

            # ---- AllGather o; full proj; residual
            agi = dr.tile([192, TP], MM_DT, tag="agi")
            ago = dr.tile([768, TP], MM_DT, tag="ago")
            nc.sync.dma_start(agi.rearrange("(h d) t -> d h t", d=DH), oown[:])
            nc.gpsimd.collective_compute("AllGather", ALU.bypass, replica_groups=G4,
                                         ins=[agi.opt()], outs=[ago.opt()])
            ofull = sb.tile([128, 6, TP], MM_DT, tag="ofull")
            nc.sync.dma_start(ofull[:], ago.rearrange("(k p) t -> p k t", p=128))
            for m in range(6):
                wpj = wchunk(d_wproj, m, "wpj")
                pp = psA.tile([128, TP], F32, tag="pbig")
                for k in range(6):
                    for (n0, nw) in NCH:
                        nc.tensor.matmul(pp[:, n0:n0 + nw], wpj[:, k, :],
                                         ofull[:, k, n0:n0 + nw], start=(k == 0), stop=(k == 5))
                nc.vector.scalar_tensor_tensor(out=x[:, m, :], in0=pp[:],
                                               scalar=vec[:, 0, m:m + 1], in1=x[:, m, :],
                                               op0=ALU.add, op1=ALU.add)

            # ---- MLP
            a2, c2 = ln_rows(x)
            a2b = bcast(a2, 128, "a1b"); c2b = bcast(c2, 128, "c1b")
            xn2 = ln_apply(x, a2b, c2b, MM_DT, tag="xn")
            hg = sb.tile([128, 6, TP], MM_DT, tag="ofull")
            for m in range(6):
                wf1 = wchunk(d_wfc1, m, "wf1")
                ph = psA.tile([128, TP], F32, tag="pbig")
                for k in range(6):
                    for (n0, nw) in NCH:
                        nc.tensor.matmul(ph[:, n0:n0 + nw], wf1[:, k, :],
                                         xn2[:, k, n0:n0 + nw], start=(k == 0), stop=(k == 5))
                nc.scalar.activation(out=hg[:, m, :], in_=ph[:], func=AF.Gelu,
                                     bias=vec[:, 5, m:m + 1], scale=1.0)
            arb_i = dr.tile([768, TP], F32, tag="arbi")
            arb_o = dr.tile([768, TP], F32, tag="arbo")
            for m in range(6):
                wf2 = wchunk(d_wfc2, m, "wf2")
                pf = psA.tile([128, TP], F32, tag="pbig")
                for k in range(6):
                    for (n0, nw) in NCH:
                        nc.tensor.matmul(pf[:, n0:n0 + nw], wf2[:, k, :],
                                         hg[:, k, n0:n0 + nw], start=(k == 0), stop=(k == 5))
                m2 = sb.tile([128, TP], F32, tag="strm", bufs=2)
                nc.scalar.activation(out=m2[:], in_=pf[:], func=AF.Copy)
                nc.sync.dma_start(arb_i.rearrange("(k p) t -> p k t", p=128)[:, m, :], m2[:])
            nc.gpsimd.collective_compute("AllReduce", ALU.add, replica_groups=G4,
                                         ins=[arb_i.opt()], outs=[arb_o.opt()])
            for m in range(6):
                ars = sb.tile([128, TP], F32, tag="strm2", bufs=2)
                nc.sync.dma_start(ars[:], arb_o.rearrange("(k p) t -> p k t", p=128)[:, m, :])
                nc.vector.scalar_tensor_tensor(out=x[:, m, :], in0=ars[:],
                                               scalar=vec[:, 1, m:m + 1], in1=x[:, m, :],
                                               op0=ALU.add, op1=ALU.add)

            if l in snap_slot:
                nc.sync.dma_start(
                    d_out.ap()[snap_slot[l], 0].rearrange("(k p) t -> p k t", p=128), x[:])

            if l in out_slot:
                fv = sb.tile([128, 2, 6], F32, tag="fv")
                nc.sync.dma_start(fv[:], d_fvec.ap().rearrange("v k p -> p v k"))
                af, cf = ln_rows(x)
                afb = bcast(af, 128, "a1b"); cfb = bcast(cf, 128, "c1b")
                for k in range(6):
                    t1 = sb.tile([128, TP], F32, tag="lntmp", bufs=2)
                    nc.vector.tensor_mul(t1[:], x[:, k, :], afb[:])
                    nc.vector.tensor_add(t1[:], t1[:], cfb[:])
                    lnfk = sb.tile([128, TP], F32, tag="strm", bufs=2)
                    nc.vector.tensor_scalar(out=lnfk[:], in0=t1[:],
                                            scalar1=fv[:, 0, k:k + 1], scalar2=fv[:, 1, k:k + 1],
                                            op0=ALU.mult, op1=ALU.add)
                    nc.sync.dma_start(
                        d_out.ap()[out_slot[l], 1].rearrange("(k p) t -> p k t", p=128)[:, k, :],
                        lnfk[:])
        if NL < L:
            nc.sync.dma_start(d_out.ap()[0, 0].rearrange("(k p) t -> p k t", p=128), x[:])
    nc.finalize()
    return nc


_NC_CACHE = {}


def kernel(**inputs):
    n_layers = int(inputs.pop("_n_layers", L))
    if n_layers not in _NC_CACHE:
        _NC_CACHE[n_layers] = build_nc(n_layers)
    nc = _NC_CACHE[n_layers]
    in_maps = [_prep_core_inputs(c, inputs) for c in range(NCORES)]
    res = None
    for attempt in range(3):
        try:
            res = run_bass_kernel_spmd(nc, in_maps, core_ids=list(range(NCORES)))
            break
        except Exception:
            if attempt == 2:
                raise
    outs = res.results
    ps = []
    for j in range(4):
        views = []
        for v in range(S):
            ob = outs[4 * v]["outbuf"][j]
            loc = ob[0][:, :T].T
            lnf = ob[1][:, :T].T
            views.append(np.concatenate([loc, lnf], -1))
        ps.append(np.stack(views, 0)[None])
    cam_views = []
    for v in range(S):
        x10_t0 = outs[4 * v]["outbuf"][3, 0][:, 0]
        x11_t0 = outs[4 * v]["outbuf"][4, 0][:, 0]
        cam_views.append(np.concatenate([x10_t0, x11_t0], -1))
    cam = np.stack(cam_views, 0)[None]
    out = tuple(p[:, :, 1:, :].astype(np.float32) for p in ps) + (cam.astype(np.float32),)
    return out


# revision 16
# speedup vs baseline: 1.0315x; 1.0315x over previous
"""DinoV2 backbone wrapper — 8-core Trainium2 Bass kernel.

Sharding: core c -> view v=c//4, head-group g=c%4 (heads 3g..3g+2, mlp hidden
chunk g). Feature-major activations [C=768 partitions x T tokens free].
fp32r matmuls (tokens padded 577->640, all matmul free dims even).
Per layer: AllGather(4-group) of attention head outputs -> full proj on every
core; AllReduce(4-group) of fc2 partials. Global layers additionally pair-
AllGather roped k + v between the two view-cores.

Host-side folds: LN1/LN2 scale+bias into qkv/v/fc1 weights+biases; layerscale
gamma into proj/fc2 weight columns; RoPE rotation into extra weight columns
(q_rot, k_rot); v bias applied post-attention (softmax sums to 1).
"""
import sys
import numpy as np

sys.path.insert(0, '/root/.axon_site')

import concourse.bass as bass
import concourse.bacc as bacc
import concourse.mybir as mybir
from concourse import tile
from concourse.bass_utils import run_bass_kernel_spmd
from contextlib import ExitStack

F32 = mybir.dt.float32
F32R = mybir.dt.float32r
AF = mybir.ActivationFunctionType
ALU = mybir.AluOpType

L, C, H, DH, S, N, B = 12, 768, 12, 64, 2, 577, 1
EPS = 1e-6
T = 577
TP = 640            # padded tokens (5*128)
KC = 5              # local key chunks
KCG = 10            # global key chunks
NCH = [(0, 512), (512, 128)]
NCORES = 8
MM_DT = F32R        # big-matmul dtype


def _rot_perm_sign():
    idx = np.zeros(DH, np.int64)
    sgn = np.zeros(DH, np.float32)
    for base in (0, 32):
        for d in range(16):
            idx[base + d] = base + d + 16
            sgn[base + d] = -1.0
            idx[base + 16 + d] = base + d
            sgn[base + 16 + d] = 1.0
    return idx, sgn


def _prep_core_inputs(c, inputs):
    v, g = c // 4, c % 4
    heads = [3 * g, 3 * g + 1, 3 * g + 2]
    x = np.asarray(inputs['x'])
    cam = np.asarray(inputs['camera_token'])
    qkv_w = np.asarray(inputs['qkv_w']); qkv_b = np.asarray(inputs['qkv_b'])
    qn = np.asarray(inputs['q_norm_w']); kn = np.asarray(inputs['k_norm_w'])
    pw = np.asarray(inputs['proj_w']); pb = np.asarray(inputs['proj_b'])
    g1 = np.asarray(inputs['ls1_gamma']); g2 = np.asarray(inputs['ls2_gamma'])
    n1w = np.asarray(inputs['norm1_w']); n1b = np.asarray(inputs['norm1_b'])
    n2w = np.asarray(inputs['norm2_w']); n2b = np.asarray(inputs['norm2_b'])
    f1w = np.asarray(inputs['fc1_w']); f1b = np.asarray(inputs['fc1_b'])
    f2w = np.asarray(inputs['fc2_w']); f2b = np.asarray(inputs['fc2_b'])
    fnw = np.asarray(inputs['final_norm_w']); fnb = np.asarray(inputs['final_norm_b'])
    cosl = np.asarray(inputs['rope_cos_local']); sinl = np.asarray(inputs['rope_sin_local'])
    cosg = np.asarray(inputs['rope_cos_global']); sing = np.asarray(inputs['rope_sin_global'])
    kvl = np.asarray(inputs['key_valid_local']); kvg = np.asarray(inputs['key_valid_global'])

    ridx, rsgn = _rot_perm_sign()

    xv = np.array(x[0, v])
    xv[0] = cam[0, v]
    x0 = np.zeros((C, TP), np.float32)
    x0[:, :T] = xv.T

    wqkv = np.zeros((L, C, 768), np.float32)
    wv = np.zeros((L, C, 198), np.float32)
    wproj = np.zeros((L, C, C), np.float32)
    wfc1 = np.zeros((L, C, C), np.float32)
    wfc2 = np.zeros((L, C, C), np.float32)
    vecs = np.zeros((L, 6, 6, 128), np.float32)
    bvv = np.zeros((L, 3, DH), np.float32)
    qb12 = np.zeros((L, 12, DH), np.float32)
    qnv = np.zeros((L, DH), np.float32); knv = np.zeros((L, DH), np.float32)

    def chan(vec):
        return vec.reshape(6, 128)

    for l in range(L):
        hrows = np.concatenate([np.arange(h * DH, (h + 1) * DH) for h in heads])
        q_w = qkv_w[l][hrows, :]; k_w = qkv_w[l][C + hrows, :]; v_w = qkv_w[l][2 * C + hrows, :]
        q_b = qkv_b[l][hrows]; k_b = qkv_b[l][C + hrows]; v_b = qkv_b[l][2 * C + hrows]

        def rot(mat, bias):
            m2 = np.zeros_like(mat); b2 = np.zeros_like(bias)
            for hh in range(3):
                blk = mat[hh * DH:(hh + 1) * DH]; bb = bias[hh * DH:(hh + 1) * DH]
                m2[hh * DH:(hh + 1) * DH] = rsgn[:, None] * blk[ridx]
                b2[hh * DH:(hh + 1) * DH] = rsgn * bb[ridx]
            return m2, b2
        qr_w, qr_b = rot(q_w, q_b)
        kr_w, kr_b = rot(k_w, k_b)
        big = np.concatenate([q_w, k_w, qr_w, kr_w], 0)
        bigb = np.concatenate([q_b, k_b, qr_b, kr_b], 0)
        wqkv[l] = (big * n1w[l][None, :]).T
        bqkv = bigb + big @ n1b[l]
        # v: token-major matmul, no bias in matmul (applied post-attention)
        vv = np.zeros((198, C), np.float32)
        for hh in range(3):
            vv[hh * 66:hh * 66 + DH] = v_w[hh * DH:(hh + 1) * DH]
        wv[l] = (vv * n1w[l][None, :]).T
        vbe = v_b + v_w @ n1b[l]          # effective per-channel v bias [192]
        bvv[l] = vbe.reshape(3, DH)
        wproj[l] = pw[l].T * g1[l][None, :]        # fold ls1 into proj columns
        f1 = f1w[l][768 * g:768 * (g + 1)]
        wfc1[l] = (f1 * n2w[l][None, :]).T
        bfc1 = f1b[l][768 * g:768 * (g + 1)] + f1 @ n2b[l]
        wfc2[l] = f2w[l][:, 768 * g:768 * (g + 1)].T * g2[l][None, :]  # fold ls2
        vecs[l, 0] = chan(g1[l] * pb[l])
        vecs[l, 1] = chan(g2[l] * f2b[l])
        vecs[l, 4] = chan(bqkv)
        vecs[l, 5] = chan(bfc1)
        qb12[l] = bqkv.reshape(12, 64)
        qnv[l] = qn[l]; knv[l] = kn[l]

    fvec = np.stack([chan(fnw), chan(fnb)], 0)

    def tabT(tab):
        out = np.zeros((DH, TP), np.float32); out[:, :T] = tab.T; return out
    cosl_t = tabT(cosl); sinl_t = tabT(sinl)
    cosg_t = tabT(cosg[v * T:(v + 1) * T]); sing_t = tabT(sing[v * T:(v + 1) * T])

    def maskify(kv, nkc):
        m = np.full(nkc * 128, -10000.0, np.float32)
        m[:kv.shape[0]] = (1.0 - kv) * -10000.0
        return m.reshape(nkc, 128)
    ml = maskify(kvl[v], KC)
    mg = np.concatenate([maskify(kvg[0, :T], KC), maskify(kvg[0, T:], KC)], 0)

    return {
        "x0": x0, "wqkv": wqkv, "wv": wv, "wproj": wproj, "wfc1": wfc1,
        "wfc2": wfc2, "vecs": vecs, "fvec": fvec, "bvv": bvv,
        "qnv": qnv, "knv": knv, "qb12": qb12,
        "cosl": cosl_t, "sinl": sinl_t, "cosg": cosg_t, "sing": sing_t,
        "ml": ml, "mg": mg,
    }


def build_nc(n_layers=L):
    nc = bacc.Bacc("TRN2", target_bir_lowering=False, debug=False, num_devices=NCORES)
    NL = n_layers
    d_x0 = nc.dram_tensor("x0", [C, TP], F32, kind="ExternalInput")
    d_wqkv = nc.dram_tensor("wqkv", [L, C, 768], MM_DT, kind="ExternalInput")
    d_wv = nc.dram_tensor("wv", [L, C, 198], MM_DT, kind="ExternalInput")
    d_wproj = nc.dram_tensor("wproj", [L, C, C], MM_DT, kind="ExternalInput")
    d_wfc1 = nc.dram_tensor("wfc1", [L, C, C], MM_DT, kind="ExternalInput")
    d_wfc2 = nc.dram_tensor("wfc2", [L, C, C], MM_DT, kind="ExternalInput")
    d_vecs = nc.dram_tensor("vecs", [L, 6, 6, 128], F32, kind="ExternalInput")
    d_fvec = nc.dram_tensor("fvec", [2, 6, 128], F32, kind="ExternalInput")
    d_bvv = nc.dram_tensor("bvv", [L, 3, DH], F32, kind="ExternalInput")
    d_qnv = nc.dram_tensor("qnv", [L, DH], F32, kind="ExternalInput")
    d_qb12 = nc.dram_tensor("qb12", [L, 12, DH], F32, kind="ExternalInput")
    d_knv = nc.dram_tensor("knv", [L, DH], F32, kind="ExternalInput")
    d_cosl = nc.dram_tensor("cosl", [DH, TP], F32, kind="ExternalInput")
    d_sinl = nc.dram_tensor("sinl", [DH, TP], F32, kind="ExternalInput")
    d_cosg = nc.dram_tensor("cosg", [DH, TP], F32, kind="ExternalInput")
    d_sing = nc.dram_tensor("sing", [DH, TP], F32, kind="ExternalInput")
    d_ml = nc.dram_tensor("ml", [KC, 128], F32, kind="ExternalInput")
    d_mg = nc.dram_tensor("mg", [KCG, 128], F32, kind="ExternalInput")
    d_out = nc.dram_tensor("outbuf", [5, 2, C, TP], F32, kind="ExternalOutput")

    G4 = [[0, 1, 2, 3], [4, 5, 6, 7]]
    G2 = [[0, 4], [1, 5], [2, 6], [3, 7]]
    KSZ = DH * 3 * TP
    VSZ = 128 * KC * 198
    KV_ELE = KSZ + VSZ

    out_slot = {2: 0, 5: 1, 8: 2, 11: 3}
    snap_slot = {2: 0, 4: 1, 8: 2, 10: 3, 11: 4}

    with tile.TileContext(nc) as tc, ExitStack() as ctx:
        sb = ctx.enter_context(tc.tile_pool(name="sb", bufs=1))
        wp = ctx.enter_context(tc.tile_pool(name="wp", bufs=2))
        psA = ctx.enter_context(tc.tile_pool(name="psA", bufs=2, space="PSUM"))
        psB = ctx.enter_context(tc.tile_pool(name="psB", bufs=2, space="PSUM"))
        dr = ctx.enter_context(tc.tile_pool(name="dr", bufs=2, space="DRAM"))

        x = sb.tile([128, 6, TP], F32, tag="x")
        nc.vector.memset(x[:], 0.0)
        nc.sync.dma_start(x[:], d_x0.ap().rearrange("(k p) t -> p k t", p=128))

        masks = sb.tile([128, KC + KCG], F32, tag="masks")
        nc.sync.dma_start(masks[:, 0:KC], d_ml.ap().rearrange("k p -> p k"))
        nc.sync.dma_start(masks[:, KC:], d_mg.ap().rearrange("k p -> p k"))
        ones1 = sb.tile([128, 2], F32, tag="ones1")
        nc.vector.memset(ones1[:], 1.0)
        eps1 = sb.tile([1, 1], F32, tag="eps1")
        nc.vector.memset(eps1[:], EPS)
        ones5 = sb.tile([128, KC, 1], F32, tag="ones5")
        nc.vector.memset(ones5[:], 1.0)

        def ln_rows(src6):
            p1 = psB.tile([1, TP], F32, tag="pav")
            p2 = psB.tile([1, TP], F32, tag="pav")
            for k in range(6):
                sq = sb.tile([128, TP], F32, tag="lnsq")
                nc.scalar.activation(out=sq[:], in_=src6[:, k, :], func=AF.Square)
                for (n0, nw) in NCH:
                    nc.tensor.matmul(p1[:, n0:n0 + nw], ones1[:, 0:1], src6[:, k, n0:n0 + nw],
                                     start=(k == 0), stop=(k == 5))
                    nc.tensor.matmul(p2[:, n0:n0 + nw], ones1[:, 0:1], sq[:, n0:n0 + nw],
                                     start=(k == 0), stop=(k == 5))
            mu = sb.tile([1, TP], F32, tag="rows", bufs=4)
            nc.scalar.mul(mu[:], p1[:], 1.0 / C)
            var = sb.tile([1, TP], F32, tag="rows", bufs=4)
            nc.vector.tensor_mul(var[:], mu[:], mu[:])
            nc.vector.scalar_tensor_tensor(out=var[:], in0=p2[:], scalar=1.0 / C,
                                           in1=var[:], op0=ALU.mult, op1=ALU.subtract)
            a = sb.tile([1, TP], F32, tag="rows", bufs=4)
            nc.scalar.activation(out=a[:], in_=var[:], func=AF.Sqrt, bias=eps1[:])
            nc.vector.reciprocal(a[:], a[:])
            cr = sb.tile([1, TP], F32, tag="rows", bufs=4)
            nc.vector.tensor_mul(cr[:], mu[:], a[:])
            nc.scalar.mul(cr[:], cr[:], -1.0)
            return a, cr

        def bcast(row, parts, tag):
            b = sb.tile([parts, TP], F32, tag=tag)
            nc.gpsimd.partition_broadcast(b[:], row[:])
            return b

        def ln_apply(src, a_b, c_b, out_dt, wrow=None, brow=None, tag="xn"):
            o = sb.tile([128, 6, TP], out_dt, tag=tag)
            for k in range(6):
                if wrow is None:
                    nc.vector.tensor_mul(o[:, k, :], src[:, k, :], a_b[:])
                    nc.vector.tensor_add(o[:, k, :], o[:, k, :], c_b[:])
                else:
                    t1 = sb.tile([128, TP], F32, tag="lntmp", bufs=2)
                    nc.vector.tensor_mul(t1[:], src[:, k, :], a_b[:])
                    nc.vector.tensor_add(t1[:], t1[:], c_b[:])
                    nc.vector.tensor_scalar(out=o[:, k, :], in0=t1[:],
                                            scalar1=wrow[:, k:k + 1], scalar2=brow[:, k:k + 1],
                                            op0=ALU.mult, op1=ALU.add)
            return o

        for l in range(NL):
            is_global = (l % 2 == 1)
            nkc = KCG if is_global else KC
            moff = KC if is_global else 0
            tabs = sb.tile([DH, 2, TP], F32, tag="tabs", bufs=2)
            nc.sync.dma_start(tabs[:, 0, :], (d_cosg if is_global else d_cosl).ap())
            nc.sync.dma_start(tabs[:, 1, :], (d_sing if is_global else d_sinl).ap())
            ctab = tabs[:, 0, :]
            stab = tabs[:, 1, :]

            def wchunk(dram, m, tag):
                wt = wp.tile([128, 6, 128], MM_DT, tag=tag, bufs=2)
                nc.sync.dma_start(wt[:], dram.ap()[l].rearrange(
                    "(k p) m -> p k m", p=128)[:, :, m * 128:(m + 1) * 128])
                return wt
            wvt = wp.tile([128, 6, 198], MM_DT, tag="wvt")
            nc.sync.dma_start(wvt[:], d_wv.ap()[l].rearrange("(k p) m -> p k m", p=128))
            vec = wp.tile([128, 6, 6], F32, tag="vec")
            nc.sync.dma_start(vec[:], d_vecs.ap()[l].rearrange("v k p -> p v k"))
            vbt = wp.tile([DH, 3], F32, tag="vbt")
            nc.sync.dma_start(vbt[:], d_bvv.ap()[l].rearrange("h d -> d h"))
            qb = wp.tile([DH, 12], F32, tag="qb")
            nc.sync.dma_start(qb[:], d_qb12.ap()[l].rearrange("j d -> d j"))
            qkn = wp.tile([DH, 2], F32, tag="qkn")
            nc.sync.dma_start(qkn[:, 0:1], d_qnv.ap()[l].rearrange("(d o) -> d o", o=1))
            nc.sync.dma_start(qkn[:, 1:2], d_knv.ap()[l].rearrange("(d o) -> d o", o=1))

            # ---- LN1 + qkv
            a1, c1 = ln_rows(x)
            a1b = bcast(a1, 128, "a1b"); c1b = bcast(c1, 128, "c1b")
            xn = ln_apply(x, a1b, c1b, MM_DT, tag="xn")
            q_all = sb.tile([DH, 3, TP], F32, tag="q_all")
            k_all = sb.tile([DH, 3, TP], F32, tag="k_all")
            qr_all = sb.tile([DH, 3, TP], F32, tag="qr_all")
            kr_all = sb.tile([DH, 3, TP], F32, tag="kr_all")
            splits = (q_all, k_all, qr_all, kr_all)
            for m in range(6):
                wq = wchunk(d_wqkv, m, "wq")
                pq = psA.tile([128, TP], F32, tag="pbig")
                for k in range(6):
                    for (n0, nw) in NCH:
                        nc.tensor.matmul(pq[:, n0:n0 + nw], wq[:, k, :],
                                         xn[:, k, n0:n0 + nw], start=(k == 0), stop=(k == 5))
                for half in range(2):
                    j = 2 * m + half
                    dst = splits[j // 3]
                    nc.vector.tensor_scalar(out=dst[:, j % 3, :],
                                            in0=pq[half * DH:(half + 1) * DH, :],
                                            scalar1=qb[:, j:j + 1], scalar2=None, op0=ALU.add)
            # v token-major
            vsb = sb.tile([128, KC, 198], MM_DT, tag="vg")
            for tcn in range(KC):
                pv = psA.tile([128, 198], F32, tag="pbig")
                for k in range(6):
                    nc.tensor.matmul(pv[:], xn[:, k, tcn * 128:(tcn + 1) * 128],
                                     wvt[:, k, :], start=(k == 0), stop=(k == 5))
                nc.scalar.activation(out=vsb[:, tcn, :], in_=pv[:], func=AF.Copy)
            for hh in range(3):
                nc.vector.tensor_copy(out=vsb[:, :, hh * 66 + DH:hh * 66 + DH + 1],
                                      in_=ones5[:])

            qf = sb.tile([DH, 3, TP], MM_DT, tag="qf")
            kf = sb.tile([DH, 3, TP], MM_DT, tag="kg")
            for (dst, raw, rot_, nslot) in ((kf, k_all, kr_all, 1), (qf, q_all, qr_all, 0)):
                if dst is qf and is_global:
                    kvin = dr.tile([KV_ELE], MM_DT, tag="kvin")
                    kvout = dr.tile([2, KV_ELE], MM_DT, tag="kvout")
                    nc.sync.dma_start(kvin[0:KSZ].rearrange("(d h t) -> d h t", d=DH, h=3), kf[:])
                    nc.sync.dma_start(kvin[KSZ:].rearrange("(p a c) -> p a c", p=128, a=KC), vsb[:])
                    nc.gpsimd.collective_compute("AllGather", ALU.bypass, replica_groups=G2,
                                                 ins=[kvin.opt()], outs=[kvout.opt()])
                for hh in range(3):
                    src = raw[:, hh, :]
                    rsq = sb.tile([DH, TP], F32, tag="rsq")
                    nc.scalar.activation(out=rsq[:], in_=src, func=AF.Square)
                    pr = psB.tile([1, TP], F32, tag="pav")
                    for (n0, nw) in NCH:
                        nc.tensor.matmul(pr[:, n0:n0 + nw], ones1[0:DH, 0:1],
                                         rsq[:, n0:n0 + nw], start=True, stop=True)
                    rr = sb.tile([1, TP], F32, tag="rows", bufs=4)
                    nc.scalar.activation(out=rr[:], in_=pr[:], func=AF.Sqrt,
                                         bias=eps1[:], scale=1.0 / DH)
                    nc.vector.reciprocal(rr[:], rr[:])
                    rb = sb.tile([DH, TP], F32, tag="rb")
                    nc.gpsimd.partition_broadcast(rb[:], rr[:])
                    t1 = sb.tile([DH, TP], F32, tag="ropet1")
                    t2 = sb.tile([DH, TP], F32, tag="ropet2")
                    nc.vector.tensor_mul(t1[:], src, ctab)
                    nc.vector.tensor_mul(t2[:], rot_[:, hh, :], stab)
                    nc.vector.tensor_add(t1[:], t1[:], t2[:])
                    nc.vector.tensor_mul(t1[:], t1[:], rb[:])
                    nc.vector.tensor_scalar(out=dst[:, hh, :], in0=t1[:],
                                            scalar1=qkn[:, nslot:nslot + 1], scalar2=None,
                                            op0=ALU.mult)

            if is_global:
                kg = sb.tile([DH, 3, 2 * TP], MM_DT, tag="kg")
                vg = sb.tile([128, KCG, 198], MM_DT, tag="vg")
                for vv in range(2):
                    nc.sync.dma_start(
                        kg[:, :, vv * TP:(vv + 1) * TP],
                        kvout[vv, 0:KSZ].rearrange("(d h t) -> d h t", d=DH, h=3))
                    nc.sync.dma_start(
                        vg[:, vv * KC:(vv + 1) * KC, :],
                        kvout[vv, KSZ:].rearrange("(p a c) -> p a c", p=128, a=KC))
                k_use, v_use = kg, vg
            else:
                k_use, v_use = kf, vsb

            # ---- attention
            oown = sb.tile([DH, 3, TP], MM_DT, tag="oown")
            for hh in range(3):
                pav = psB.tile([66, TP], F32, tag="pav")
                for kt in range(nkc):
                    sc = psA.tile([128, TP], F32, tag="pbig")
                    for (n0, nw) in NCH:
                        nc.tensor.matmul(sc[:, n0:n0 + nw],
                                         k_use[:, hh, kt * 128:(kt + 1) * 128],
                                         qf[:, hh, n0:n0 + nw], start=True, stop=True)
                    ex = sb.tile([128, TP], MM_DT, tag="ex", bufs=2)
                    nc.scalar.activation(out=ex[:], in_=sc[:], func=AF.Exp,
                                         bias=masks[:, moff + kt:moff + kt + 1], scale=0.125)
                    for (n0, nw) in NCH:
                        nc.tensor.matmul(pav[:, n0:n0 + nw],
                                         v_use[:, kt, hh * 66:hh * 66 + 66],
                                         ex[:, n0:n0 + nw],
                                         start=(kt == 0), stop=(kt == nkc - 1))
                den = sb.tile([1, TP], F32, tag="rows", bufs=4)
                nc.vector.reciprocal(den[:], pav[DH:DH + 1, :])
                dnb = sb.tile([DH, TP], F32, tag="dnb")
                nc.gpsimd.partition_broadcast(dnb[:], den[:])
                nc.vector.tensor_mul(oown[:, hh, :], pav[0:DH, :], dnb[:])
                nc.vector.tensor_scalar(out=oown[:, hh, :], in0=oown[:, hh, :],
                                        scalar1=vbt[:, hh:hh + 1], scalar2=None, op0=ALU.add)

            # ---- AllGather o; full proj; residual
            agi = dr.tile([192, TP], MM_DT, tag="agi")
            ago = dr.tile([768, TP], MM_DT, tag="ago")
            nc.sync.dma_start(agi.rearrange("(h d) t -> d h t", d=DH), oown[:])
            nc.gpsimd.collective_compute("AllGather", ALU.bypass, replica_groups=G4,
                                         ins=[agi.opt()], outs=[ago.opt()])
            ofull = sb.tile([128, 6, TP], MM_DT, tag="ofull")
            nc.sync.dma_start(ofull[:], ago.rearrange("(k p) t -> p k t", p=128))
            for m in range(6):
                wpj = wchunk(d_wproj, m, "wpj")
                pp = psA.tile([128, TP], F32, tag="pbig")
                for k in range(6):
                    for (n0, nw) in NCH:
                        nc.tensor.matmul(pp[:, n0:n0 + nw], wpj[:, k, :],
                                         ofull[:, k, n0:n0 + nw], start=(k == 0), stop=(k == 5))
                nc.vector.scalar_tensor_tensor(out=x[:, m, :], in0=pp[:],
                                               scalar=vec[:, 0, m:m + 1], in1=x[:, m, :],
                                               op0=ALU.add, op1=ALU.add)

            # ---- MLP
            a2, c2 = ln_rows(x)
            a2b = bcast(a2, 128, "a1b"); c2b = bcast(c2, 128, "c1b")
            xn2 = ln_apply(x, a2b, c2b, MM_DT, tag="xn")
            hg = sb.tile([128, 6, TP], MM_DT, tag="ofull")
            for m in range(6):
                wf1 = wchunk(d_wfc1, m, "wf1")
                ph = psA.tile([128, TP], F32, tag="pbig")
                for k in range(6):
                    for (n0, nw) in NCH:
                        nc.tensor.matmul(ph[:, n0:n0 + nw], wf1[:, k, :],
                                         xn2[:, k, n0:n0 + nw], start=(k == 0), stop=(k == 5))
                nc.scalar.activation(out=hg[:, m, :], in_=ph[:], func=AF.Gelu,
                                     bias=vec[:, 5, m:m + 1], scale=1.0)
            arb_i = dr.tile([768, TP], F32, tag="arbi")
            arb_o = dr.tile([768, TP], F32, tag="arbo")
            for m in range(6):
                wf2 = wchunk(d_wfc2, m, "wf2")
                pf = psA.tile([128, TP], F32, tag="pbig")
                for k in range(6):
                    for (n0, nw) in NCH:
                        nc.tensor.matmul(pf[:, n0:n0 + nw], wf2[:, k, :],
                                         hg[:, k, n0:n0 + nw], start=(k == 0), stop=(k == 5))
                m2 = sb.tile([128, TP], F32, tag="strm", bufs=2)
                nc.scalar.activation(out=m2[:], in_=pf[:], func=AF.Copy)
                nc.sync.dma_start(arb_i.rearrange("(k p) t -> p k t", p=128)[:, m, :], m2[:])
            nc.gpsimd.collective_compute("AllReduce", ALU.add, replica_groups=G4,
                                         ins=[arb_i.opt()], outs=[arb_o.opt()])
            for m in range(6):
                ars = sb.tile([128, TP], F32, tag="strm2", bufs=2)
                nc.sync.dma_start(ars[:], arb_o.rearrange("(k p) t -> p k t", p=128)[:, m, :])
                nc.vector.scalar_tensor_tensor(out=x[:, m, :], in0=ars[:],
                                               scalar=vec[:, 1, m:m + 1], in1=x[:, m, :],
                                               op0=ALU.add, op1=ALU.add)

            if l in snap_slot:
                nc.sync.dma_start(
                    d_out.ap()[snap_slot[l], 0].rearrange("(k p) t -> p k t", p=128), x[:])

            if l in out_slot:
                fv = sb.tile([128, 2, 6], F32, tag="fv")
                nc.sync.dma_start(fv[:], d_fvec.ap().rearrange("v k p -> p v k"))
                af, cf = ln_rows(x)
                afb = bcast(af, 128, "a1b"); cfb = bcast(cf, 128, "c1b")
                for k in range(6):
                    t1 = sb.tile([128, TP], F32, tag="lntmp", bufs=2)
                    nc.vector.tensor_mul(t1[:], x[:, k, :], afb[:])
                    nc.vector.tensor_add(t1[:], t1[:], cfb[:])
                    lnfk = sb.tile([128, TP], F32, tag="strm", bufs=2)
                    nc.vector.tensor_scalar(out=lnfk[:], in0=t1[:],
                                            scalar1=fv[:, 0, k:k + 1], scalar2=fv[:, 1, k:k + 1],
                                            op0=ALU.mult, op1=ALU.add)
                    nc.sync.dma_start(
                        d_out.ap()[out_slot[l], 1].rearrange("(k p) t -> p k t", p=128)[:, k, :],
                        lnfk[:])
        if NL < L:
            nc.sync.dma_start(d_out.ap()[0, 0].rearrange("(k p) t -> p k t", p=128), x[:])
    nc.finalize()
    return nc


_NC_CACHE = {}


def kernel(**inputs):
    n_layers = int(inputs.pop("_n_layers", L))
    if n_layers not in _NC_CACHE:
        _NC_CACHE[n_layers] = build_nc(n_layers)
    nc = _NC_CACHE[n_layers]
    in_maps = [_prep_core_inputs(c, inputs) for c in range(NCORES)]
    res = None
    for attempt in range(3):
        try:
            res = run_bass_kernel_spmd(nc, in_maps, core_ids=list(range(NCORES)))
            break
        except Exception:
            if attempt == 2:
                raise
    outs = res.results
    ps = []
    for j in range(4):
        views = []
        for v in range(S):
            ob = outs[4 * v]["outbuf"][j]
            loc = ob[0][:, :T].T
            lnf = ob[1][:, :T].T
            views.append(np.concatenate([loc, lnf], -1))
        ps.append(np.stack(views, 0)[None])
    cam_views = []
    for v in range(S):
        x10_t0 = outs[4 * v]["outbuf"][3, 0][:, 0]
        x11_t0 = outs[4 * v]["outbuf"][4, 0][:, 0]
        cam_views.append(np.concatenate([x10_t0, x11_t0], -1))
    cam = np.stack(cam_views, 0)[None]
    out = tuple(p[:, :, 1:, :].astype(np.float32) for p in ps) + (cam.astype(np.float32),)
    return out


# revision 17
# speedup vs baseline: 1.0552x; 1.0230x over previous
"""DinoV2 backbone wrapper — 8-core Trainium2 Bass kernel.

Sharding: core c -> view v=c//4, head-group g=c%4 (heads 3g..3g+2, mlp hidden
chunk g). Feature-major activations [C=768 partitions x T tokens free].
fp32r matmuls (tokens padded 577->640, all matmul free dims even).
Per layer: AllGather(4-group) of attention head outputs -> full proj on every
core; AllReduce(4-group) of fc2 partials. Global layers additionally pair-
AllGather roped k + v between the two view-cores.

Host-side folds: LN1/LN2 scale+bias into qkv/v/fc1 weights+biases; layerscale
gamma into proj/fc2 weight columns; RoPE rotation into extra weight columns
(q_rot, k_rot); v bias applied post-attention (softmax sums to 1).
"""
import sys
import numpy as np

sys.path.insert(0, '/root/.axon_site')

import concourse.bass as bass
import concourse.bacc as bacc
import concourse.mybir as mybir
from concourse import tile
from concourse.bass_utils import run_bass_kernel_spmd
from contextlib import ExitStack

F32 = mybir.dt.float32
F32R = mybir.dt.float32r
AF = mybir.ActivationFunctionType
ALU = mybir.AluOpType

L, C, H, DH, S, N, B = 12, 768, 12, 64, 2, 577, 1
EPS = 1e-6
T = 577
TP = 640            # padded tokens (5*128)
KC = 5              # local key chunks
KCG = 10            # global key chunks
NCH = [(0, 512), (512, 128)]
NCORES = 8
MM_DT = F32R        # big-matmul dtype


def _rot_perm_sign():
    idx = np.zeros(DH, np.int64)
    sgn = np.zeros(DH, np.float32)
    for base in (0, 32):
        for d in range(16):
            idx[base + d] = base + d + 16
            sgn[base + d] = -1.0
            idx[base + 16 + d] = base + d
            sgn[base + 16 + d] = 1.0
    return idx, sgn


def _prep_core_inputs(c, inputs):
    v, g = c // 4, c % 4
    heads = [3 * g, 3 * g + 1, 3 * g + 2]
    x = np.asarray(inputs['x'])
    cam = np.asarray(inputs['camera_token'])
    qkv_w = np.asarray(inputs['qkv_w']); qkv_b = np.asarray(inputs['qkv_b'])
    qn = np.asarray(inputs['q_norm_w']); kn = np.asarray(inputs['k_norm_w'])
    pw = np.asarray(inputs['proj_w']); pb = np.asarray(inputs['proj_b'])
    g1 = np.asarray(inputs['ls1_gamma']); g2 = np.asarray(inputs['ls2_gamma'])
    n1w = np.asarray(inputs['norm1_w']); n1b = np.asarray(inputs['norm1_b'])
    n2w = np.asarray(inputs['norm2_w']); n2b = np.asarray(inputs['norm2_b'])
    f1w = np.asarray(inputs['fc1_w']); f1b = np.asarray(inputs['fc1_b'])
    f2w = np.asarray(inputs['fc2_w']); f2b = np.asarray(inputs['fc2_b'])
    fnw = np.asarray(inputs['final_norm_w']); fnb = np.asarray(inputs['final_norm_b'])
    cosl = np.asarray(inputs['rope_cos_local']); sinl = np.asarray(inputs['rope_sin_local'])
    cosg = np.asarray(inputs['rope_cos_global']); sing = np.asarray(inputs['rope_sin_global'])
    kvl = np.asarray(inputs['key_valid_local']); kvg = np.asarray(inputs['key_valid_global'])

    ridx, rsgn = _rot_perm_sign()

    xv = np.array(x[0, v])
    xv[0] = cam[0, v]
    x0 = np.zeros((C, TP), np.float32)
    x0[:, :T] = xv.T

    wqkv = np.zeros((L, C, 768), np.float32)
    wv = np.zeros((L, C, 198), np.float32)
    wproj = np.zeros((L, C, C), np.float32)
    wfc1 = np.zeros((L, C, C), np.float32)
    wfc2 = np.zeros((L, C, C), np.float32)
    vecs = np.zeros((L, 6, 6, 128), np.float32)
    bvv = np.zeros((L, 3, DH), np.float32)
    qb12 = np.zeros((L, 12, DH), np.float32)
    qnv = np.zeros((L, DH), np.float32); knv = np.zeros((L, DH), np.float32)

    def chan(vec):
        return vec.reshape(6, 128)

    for l in range(L):
        hrows = np.concatenate([np.arange(h * DH, (h + 1) * DH) for h in heads])
        q_w = qkv_w[l][hrows, :]; k_w = qkv_w[l][C + hrows, :]; v_w = qkv_w[l][2 * C + hrows, :]
        q_b = qkv_b[l][hrows]; k_b = qkv_b[l][C + hrows]; v_b = qkv_b[l][2 * C + hrows]

        def rot(mat, bias):
            m2 = np.zeros_like(mat); b2 = np.zeros_like(bias)
            for hh in range(3):
                blk = mat[hh * DH:(hh + 1) * DH]; bb = bias[hh * DH:(hh + 1) * DH]
                m2[hh * DH:(hh + 1) * DH] = rsgn[:, None] * blk[ridx]
                b2[hh * DH:(hh + 1) * DH] = rsgn * bb[ridx]
            return m2, b2
        qr_w, qr_b = rot(q_w, q_b)
        kr_w, kr_b = rot(k_w, k_b)
        big = np.concatenate([q_w, k_w, qr_w, kr_w], 0)
        bigb = np.concatenate([q_b, k_b, qr_b, kr_b], 0)
        wqkv[l] = (big * n1w[l][None, :]).T
        bqkv = bigb + big @ n1b[l]
        # v: token-major matmul, no bias in matmul (applied post-attention)
        vv = np.zeros((198, C), np.float32)
        for hh in range(3):
            vv[hh * 66:hh * 66 + DH] = v_w[hh * DH:(hh + 1) * DH]
        wv[l] = (vv * n1w[l][None, :]).T
        vbe = v_b + v_w @ n1b[l]          # effective per-channel v bias [192]
        bvv[l] = vbe.reshape(3, DH)
        wproj[l] = pw[l].T * g1[l][None, :]        # fold ls1 into proj columns
        f1 = f1w[l][768 * g:768 * (g + 1)]
        wfc1[l] = (f1 * n2w[l][None, :]).T
        bfc1 = f1b[l][768 * g:768 * (g + 1)] + f1 @ n2b[l]
        wfc2[l] = f2w[l][:, 768 * g:768 * (g + 1)].T * g2[l][None, :]  # fold ls2
        vecs[l, 0] = chan(g1[l] * pb[l])
        vecs[l, 1] = chan(g2[l] * f2b[l])
        vecs[l, 4] = chan(bqkv)
        vecs[l, 5] = chan(bfc1)
        qb12[l] = bqkv.reshape(12, 64)
        qnv[l] = qn[l]; knv[l] = kn[l]

    fvec = np.stack([chan(fnw), chan(fnb)], 0)

    def tabT(tab):
        out = np.zeros((DH, TP), np.float32); out[:, :T] = tab.T; return out
    cosl_t = tabT(cosl); sinl_t = tabT(sinl)
    cosg_t = tabT(cosg[v * T:(v + 1) * T]); sing_t = tabT(sing[v * T:(v + 1) * T])

    def maskify(kv, nkc):
        m = np.full(nkc * 128, -10000.0, np.float32)
        m[:kv.shape[0]] = (1.0 - kv) * -10000.0
        return m.reshape(nkc, 128)
    ml = maskify(kvl[v], KC)
    mg = np.concatenate([maskify(kvg[0, :T], KC), maskify(kvg[0, T:], KC)], 0)

    def chunked(w):  # [L, 768in, 768out] -> [L, 6m, 128p, 6k, 128c]
        return np.ascontiguousarray(
            w.reshape(L, 6, 128, 6, 128).transpose(0, 3, 2, 1, 4))
    wqkv = chunked(wqkv); wproj = chunked(wproj)
    wfc1 = chunked(wfc1); wfc2 = chunked(wfc2)
    return {
        "x0": x0, "wqkv": wqkv, "wv": wv, "wproj": wproj, "wfc1": wfc1,
        "wfc2": wfc2, "vecs": vecs, "fvec": fvec, "bvv": bvv,
        "qnv": qnv, "knv": knv, "qb12": qb12,
        "cosl": cosl_t, "sinl": sinl_t, "cosg": cosg_t, "sing": sing_t,
        "ml": ml, "mg": mg,
    }


def build_nc(n_layers=L):
    nc = bacc.Bacc("TRN2", target_bir_lowering=False, debug=False, num_devices=NCORES)
    NL = n_layers
    d_x0 = nc.dram_tensor("x0", [C, TP], F32, kind="ExternalInput")
    d_wqkv = nc.dram_tensor("wqkv", [L, 6, 128, 6, 128], MM_DT, kind="ExternalInput")
    d_wv = nc.dram_tensor("wv", [L, C, 198], MM_DT, kind="ExternalInput")
    d_wproj = nc.dram_tensor("wproj", [L, 6, 128, 6, 128], MM_DT, kind="ExternalInput")
    d_wfc1 = nc.dram_tensor("wfc1", [L, 6, 128, 6, 128], MM_DT, kind="ExternalInput")
    d_wfc2 = nc.dram_tensor("wfc2", [L, 6, 128, 6, 128], MM_DT, kind="ExternalInput")
    d_vecs = nc.dram_tensor("vecs", [L, 6, 6, 128], F32, kind="ExternalInput")
    d_fvec = nc.dram_tensor("fvec", [2, 6, 128], F32, kind="ExternalInput")
    d_bvv = nc.dram_tensor("bvv", [L, 3, DH], F32, kind="ExternalInput")
    d_qnv = nc.dram_tensor("qnv", [L, DH], F32, kind="ExternalInput")
    d_qb12 = nc.dram_tensor("qb12", [L, 12, DH], F32, kind="ExternalInput")
    d_knv = nc.dram_tensor("knv", [L, DH], F32, kind="ExternalInput")
    d_cosl = nc.dram_tensor("cosl", [DH, TP], F32, kind="ExternalInput")
    d_sinl = nc.dram_tensor("sinl", [DH, TP], F32, kind="ExternalInput")
    d_cosg = nc.dram_tensor("cosg", [DH, TP], F32, kind="ExternalInput")
    d_sing = nc.dram_tensor("sing", [DH, TP], F32, kind="ExternalInput")
    d_ml = nc.dram_tensor("ml", [KC, 128], F32, kind="ExternalInput")
    d_mg = nc.dram_tensor("mg", [KCG, 128], F32, kind="ExternalInput")
    d_out = nc.dram_tensor("outbuf", [5, 2, C, TP], F32, kind="ExternalOutput")

    G4 = [[0, 1, 2, 3], [4, 5, 6, 7]]
    G2 = [[0, 4], [1, 5], [2, 6], [3, 7]]
    KSZ = DH * 3 * TP
    VSZ = 128 * KC * 198
    KV_ELE = KSZ + VSZ

    out_slot = {2: 0, 5: 1, 8: 2, 11: 3}
    snap_slot = {2: 0, 4: 1, 8: 2, 10: 3, 11: 4}

    with tile.TileContext(nc) as tc, ExitStack() as ctx:
        sb = ctx.enter_context(tc.tile_pool(name="sb", bufs=1))
        wp = ctx.enter_context(tc.tile_pool(name="wp", bufs=2))
        psA = ctx.enter_context(tc.tile_pool(name="psA", bufs=2, space="PSUM"))
        psB = ctx.enter_context(tc.tile_pool(name="psB", bufs=2, space="PSUM"))
        dr = ctx.enter_context(tc.tile_pool(name="dr", bufs=2, space="DRAM"))

        x = sb.tile([128, 6, TP], F32, tag="x")
        nc.vector.memset(x[:], 0.0)
        nc.sync.dma_start(x[:], d_x0.ap().rearrange("(k p) t -> p k t", p=128))

        masks = sb.tile([128, KC + KCG], F32, tag="masks")
        nc.sync.dma_start(masks[:, 0:KC], d_ml.ap().rearrange("k p -> p k"))
        nc.sync.dma_start(masks[:, KC:], d_mg.ap().rearrange("k p -> p k"))
        ones1 = sb.tile([128, 2], F32, tag="ones1")
        nc.vector.memset(ones1[:], 1.0)
        eps1 = sb.tile([1, 1], F32, tag="eps1")
        nc.vector.memset(eps1[:], EPS)
        ones5 = sb.tile([128, KC, 1], F32, tag="ones5")
        nc.vector.memset(ones5[:], 1.0)

        def ln_rows(src6):
            p1 = psB.tile([1, TP], F32, tag="pav")
            p2 = psB.tile([1, TP], F32, tag="pav")
            for k in range(6):
                sq = sb.tile([128, TP], F32, tag="lnsq")
                nc.scalar.activation(out=sq[:], in_=src6[:, k, :], func=AF.Square)
                for (n0, nw) in NCH:
                    nc.tensor.matmul(p1[:, n0:n0 + nw], ones1[:, 0:1], src6[:, k, n0:n0 + nw],
                                     start=(k == 0), stop=(k == 5))
                    nc.tensor.matmul(p2[:, n0:n0 + nw], ones1[:, 0:1], sq[:, n0:n0 + nw],
                                     start=(k == 0), stop=(k == 5))
            mu = sb.tile([1, TP], F32, tag="rows", bufs=4)
            nc.scalar.mul(mu[:], p1[:], 1.0 / C)
            var = sb.tile([1, TP], F32, tag="rows", bufs=4)
            nc.vector.tensor_mul(var[:], mu[:], mu[:])
            nc.vector.scalar_tensor_tensor(out=var[:], in0=p2[:], scalar=1.0 / C,
                                           in1=var[:], op0=ALU.mult, op1=ALU.subtract)
            a = sb.tile([1, TP], F32, tag="rows", bufs=4)
            nc.scalar.activation(out=a[:], in_=var[:], func=AF.Sqrt, bias=eps1[:])
            nc.vector.reciprocal(a[:], a[:])
            cr = sb.tile([1, TP], F32, tag="rows", bufs=4)
            nc.vector.tensor_mul(cr[:], mu[:], a[:])
            nc.scalar.mul(cr[:], cr[:], -1.0)
            return a, cr

        def bcast(row, parts, tag):
            b = sb.tile([parts, TP], F32, tag=tag)
            nc.gpsimd.partition_broadcast(b[:], row[:])
            return b

        def ln_apply(src, a_b, c_b, out_dt, wrow=None, brow=None, tag="xn"):
            o = sb.tile([128, 6, TP], out_dt, tag=tag)
            for k in range(6):
                if wrow is None:
                    nc.vector.tensor_mul(o[:, k, :], src[:, k, :], a_b[:])
                    nc.vector.tensor_add(o[:, k, :], o[:, k, :], c_b[:])
                else:
                    t1 = sb.tile([128, TP], F32, tag="lntmp", bufs=2)
                    nc.vector.tensor_mul(t1[:], src[:, k, :], a_b[:])
                    nc.vector.tensor_add(t1[:], t1[:], c_b[:])
                    nc.vector.tensor_scalar(out=o[:, k, :], in0=t1[:],
                                            scalar1=wrow[:, k:k + 1], scalar2=brow[:, k:k + 1],
                                            op0=ALU.mult, op1=ALU.add)
            return o

        for l in range(NL):
            is_global = (l % 2 == 1)
            nkc = KCG if is_global else KC
            moff = KC if is_global else 0
            tabs = sb.tile([DH, 2, TP], F32, tag="tabs", bufs=2)
            nc.sync.dma_start(tabs[:, 0, :], (d_cosg if is_global else d_cosl).ap())
            nc.sync.dma_start(tabs[:, 1, :], (d_sing if is_global else d_sinl).ap())
            ctab = tabs[:, 0, :]
            stab = tabs[:, 1, :]

            def wchunk(dram, m, tag):
                wt = wp.tile([128, 6, 128], MM_DT, tag=tag, bufs=2)
                nc.sync.dma_start(wt[:], dram.ap()[l, m])
                return wt
            wvt = wp.tile([128, 6, 198], MM_DT, tag="wvt")
            nc.sync.dma_start(wvt[:], d_wv.ap()[l].rearrange("(k p) m -> p k m", p=128))
            vec = wp.tile([128, 6, 6], F32, tag="vec")
            nc.sync.dma_start(vec[:], d_vecs.ap()[l].rearrange("v k p -> p v k"))
            vbt = wp.tile([DH, 3], F32, tag="vbt")
            nc.sync.dma_start(vbt[:], d_bvv.ap()[l].rearrange("h d -> d h"))
            qb = wp.tile([DH, 12], F32, tag="qb")
            nc.sync.dma_start(qb[:], d_qb12.ap()[l].rearrange("j d -> d j"))
            qkn = wp.tile([DH, 2], F32, tag="qkn")
            nc.sync.dma_start(qkn[:, 0:1], d_qnv.ap()[l].rearrange("(d o) -> d o", o=1))
            nc.sync.dma_start(qkn[:, 1:2], d_knv.ap()[l].rearrange("(d o) -> d o", o=1))

            # ---- LN1 + qkv
            a1, c1 = ln_rows(x)
            a1b = bcast(a1, 128, "a1b"); c1b = bcast(c1, 128, "c1b")
            xn = ln_apply(x, a1b, c1b, MM_DT, tag="xn")
            q_all = sb.tile([DH, 3, TP], F32, tag="q_all")
            k_all = sb.tile([DH, 3, TP], F32, tag="k_all")
            qr_all = sb.tile([DH, 3, TP], F32, tag="qr_all")
            kr_all = sb.tile([DH, 3, TP], F32, tag="kr_all")
            splits = (q_all, k_all, qr_all, kr_all)
            for m in range(6):
                wq = wchunk(d_wqkv, m, "wq")
                pq = psA.tile([128, TP], F32, tag="pbig")
                for k in range(6):
                    for (n0, nw) in NCH:
                        nc.tensor.matmul(pq[:, n0:n0 + nw], wq[:, k, :],
                                         xn[:, k, n0:n0 + nw], start=(k == 0), stop=(k == 5))
                for half in range(2):
                    j = 2 * m + half
                    dst = splits[j // 3]
                    nc.vector.tensor_scalar(out=dst[:, j % 3, :],
                                            in0=pq[half * DH:(half + 1) * DH, :],
                                            scalar1=qb[:, j:j + 1], scalar2=None, op0=ALU.add)
            # v token-major
            vsb = sb.tile([128, KC, 198], MM_DT, tag="vg")
            for tcn in range(KC):
                pv = psA.tile([128, 198], F32, tag="pbig")
                for k in range(6):
                    nc.tensor.matmul(pv[:], xn[:, k, tcn * 128:(tcn + 1) * 128],
                                     wvt[:, k, :], start=(k == 0), stop=(k == 5))
                nc.scalar.activation(out=vsb[:, tcn, :], in_=pv[:], func=AF.Copy)
            for hh in range(3):
                nc.vector.tensor_copy(out=vsb[:, :, hh * 66 + DH:hh * 66 + DH + 1],
                                      in_=ones5[:])

            qf = sb.tile([DH, 3, TP], MM_DT, tag="qf")
            kf = sb.tile([DH, 3, TP], MM_DT, tag="kg")
            for (dst, raw, rot_, nslot) in ((kf, k_all, kr_all, 1), (qf, q_all, qr_all, 0)):
                if dst is qf and is_global:
                    kvin = dr.tile([KV_ELE], MM_DT, tag="kvin")
                    kvout = dr.tile([2, KV_ELE], MM_DT, tag="kvout")
                    nc.sync.dma_start(kvin[0:KSZ].rearrange("(d h t) -> d h t", d=DH, h=3), kf[:])
                    nc.sync.dma_start(kvin[KSZ:].rearrange("(p a c) -> p a c", p=128, a=KC), vsb[:])
                    nc.gpsimd.collective_compute("AllGather", ALU.bypass, replica_groups=G2,
                                                 ins=[kvin.opt()], outs=[kvout.opt()])
                for hh in range(3):
                    src = raw[:, hh, :]
                    rsq = sb.tile([DH, TP], F32, tag="rsq")
                    nc.scalar.activation(out=rsq[:], in_=src, func=AF.Square)
                    pr = psB.tile([1, TP], F32, tag="pav")
                    for (n0, nw) in NCH:
                        nc.tensor.matmul(pr[:, n0:n0 + nw], ones1[0:DH, 0:1],
                                         rsq[:, n0:n0 + nw], start=True, stop=True)
                    rr = sb.tile([1, TP], F32, tag="rows", bufs=4)
                    nc.scalar.activation(out=rr[:], in_=pr[:], func=AF.Sqrt,
                                         bias=eps1[:], scale=1.0 / DH)
                    nc.vector.reciprocal(rr[:], rr[:])
                    rb = sb.tile([DH, TP], F32, tag="rb")
                    nc.gpsimd.partition_broadcast(rb[:], rr[:])
                    t1 = sb.tile([DH, TP], F32, tag="ropet1")
                    t2 = sb.tile([DH, TP], F32, tag="ropet2")
                    nc.vector.tensor_mul(t1[:], src, ctab)
                    nc.vector.tensor_mul(t2[:], rot_[:, hh, :], stab)
                    nc.vector.tensor_add(t1[:], t1[:], t2[:])
                    nc.vector.tensor_mul(t1[:], t1[:], rb[:])
                    nc.vector.tensor_scalar(out=dst[:, hh, :], in0=t1[:],
                                            scalar1=qkn[:, nslot:nslot + 1], scalar2=None,
                                            op0=ALU.mult)

            if is_global:
                kg = sb.tile([DH, 3, 2 * TP], MM_DT, tag="kg")
                vg = sb.tile([128, KCG, 198], MM_DT, tag="vg")
                for vv in range(2):
                    nc.sync.dma_start(
                        kg[:, :, vv * TP:(vv + 1) * TP],
                        kvout[vv, 0:KSZ].rearrange("(d h t) -> d h t", d=DH, h=3))
                    nc.sync.dma_start(
                        vg[:, vv * KC:(vv + 1) * KC, :],
                        kvout[vv, KSZ:].rearrange("(p a c) -> p a c", p=128, a=KC))
                k_use, v_use = kg, vg
            else:
                k_use, v_use = kf, vsb

            # ---- attention
            oown = sb.tile([DH, 3, TP], MM_DT, tag="oown")
            for hh in range(3):
                pav = psB.tile([66, TP], F32, tag="pav")
                for kt in range(nkc):
                    sc = psA.tile([128, TP], F32, tag="pbig")
                    for (n0, nw) in NCH:
                        nc.tensor.matmul(sc[:, n0:n0 + nw],
                                         k_use[:, hh, kt * 128:(kt + 1) * 128],
                                         qf[:, hh, n0:n0 + nw], start=True, stop=True)
                    ex = sb.tile([128, TP], MM_DT, tag="ex", bufs=2)
                    nc.scalar.activation(out=ex[:], in_=sc[:], func=AF.Exp,
                                         bias=masks[:, moff + kt:moff + kt + 1], scale=0.125)
                    for (n0, nw) in NCH:
                        nc.tensor.matmul(pav[:, n0:n0 + nw],
                                         v_use[:, kt, hh * 66:hh * 66 + 66],
                                         ex[:, n0:n0 + nw],
                                         start=(kt == 0), stop=(kt == nkc - 1))
                den = sb.tile([1, TP], F32, tag="rows", bufs=4)
                nc.vector.reciprocal(den[:], pav[DH:DH + 1, :])
                dnb = sb.tile([DH, TP], F32, tag="dnb")
                nc.gpsimd.partition_broadcast(dnb[:], den[:])
                nc.vector.tensor_mul(oown[:, hh, :], pav[0:DH, :], dnb[:])
                nc.vector.tensor_scalar(out=oown[:, hh, :], in0=oown[:, hh, :],
                                        scalar1=vbt[:, hh:hh + 1], scalar2=None, op0=ALU.add)

            # ---- AllGather o; full proj; residual
            agi = dr.tile([192, TP], MM_DT, tag="agi")
            ago = dr.tile([768, TP], MM_DT, tag="ago")
            nc.sync.dma_start(agi.rearrange("(h d) t -> d h t", d=DH), oown[:])
            nc.gpsimd.collective_compute("AllGather", ALU.bypass, replica_groups=G4,
                                         ins=[agi.opt()], outs=[ago.opt()])
            ofull = sb.tile([128, 6, TP], MM_DT, tag="ofull")
            nc.sync.dma_start(ofull[:], ago.rearrange("(k p) t -> p k t", p=128))
            for m in range(6):
                wpj = wchunk(d_wproj, m, "wpj")
                pp = psA.tile([128, TP], F32, tag="pbig")
                for k in range(6):
                    for (n0, nw) in NCH:
                        nc.tensor.matmul(pp[:, n0:n0 + nw], wpj[:, k, :],
                                         ofull[:, k, n0:n0 + nw], start=(k == 0), stop=(k == 5))
                nc.vector.scalar_tensor_tensor(out=x[:, m, :], in0=pp[:],
                                               scalar=vec[:, 0, m:m + 1], in1=x[:, m, :],
                                               op0=ALU.add, op1=ALU.add)

            # ---- MLP
            a2, c2 = ln_rows(x)
            a2b = bcast(a2, 128, "a1b"); c2b = bcast(c2, 128, "c1b")
            xn2 = ln_apply(x, a2b, c2b, MM_DT, tag="xn")
            hg = sb.tile([128, 6, TP], MM_DT, tag="ofull")
            for m in range(6):
                wf1 = wchunk(d_wfc1, m, "wf1")
                ph = psA.tile([128, TP], F32, tag="pbig")
                for k in range(6):
                    for (n0, nw) in NCH:
                        nc.tensor.matmul(ph[:, n0:n0 + nw], wf1[:, k, :],
                                         xn2[:, k, n0:n0 + nw], start=(k == 0), stop=(k == 5))
                nc.scalar.activation(out=hg[:, m, :], in_=ph[:], func=AF.Gelu,
                                     bias=vec[:, 5, m:m + 1], scale=1.0)
            arb_i = dr.tile([768, TP], F32, tag="arbi")
            arb_o = dr.tile([768, TP], F32, tag="arbo")
            for m in range(6):
                wf2 = wchunk(d_wfc2, m, "wf2")
                pf = psA.tile([128, TP], F32, tag="pbig")
                for k in range(6):
                    for (n0, nw) in NCH:
                        nc.tensor.matmul(pf[:, n0:n0 + nw], wf2[:, k, :],
                                         hg[:, k, n0:n0 + nw], start=(k == 0), stop=(k == 5))
                m2 = sb.tile([128, TP], F32, tag="strm", bufs=2)
                nc.scalar.activation(out=m2[:], in_=pf[:], func=AF.Copy)
                nc.sync.dma_start(arb_i.rearrange("(k p) t -> p k t", p=128)[:, m, :], m2[:])
            nc.gpsimd.collective_compute("AllReduce", ALU.add, replica_groups=G4,
                                         ins=[arb_i.opt()], outs=[arb_o.opt()])
            for m in range(6):
                ars = sb.tile([128, TP], F32, tag="strm2", bufs=2)
                nc.sync.dma_start(ars[:], arb_o.rearrange("(k p) t -> p k t", p=128)[:, m, :])
                nc.vector.scalar_tensor_tensor(out=x[:, m, :], in0=ars[:],
                                               scalar=vec[:, 1, m:m + 1], in1=x[:, m, :],
                                               op0=ALU.add, op1=ALU.add)

            if l in snap_slot:
                nc.sync.dma_start(
                    d_out.ap()[snap_slot[l], 0].rearrange("(k p) t -> p k t", p=128), x[:])

            if l in out_slot:
                fv = sb.tile([128, 2, 6], F32, tag="fv")
                nc.sync.dma_start(fv[:], d_fvec.ap().rearrange("v k p -> p v k"))
                af, cf = ln_rows(x)
                afb = bcast(af, 128, "a1b"); cfb = bcast(cf, 128, "c1b")
                for k in range(6):
                    t1 = sb.tile([128, TP], F32, tag="lntmp", bufs=2)
                    nc.vector.tensor_mul(t1[:], x[:, k, :], afb[:])
                    nc.vector.tensor_add(t1[:], t1[:], cfb[:])
                    lnfk = sb.tile([128, TP], F32, tag="strm", bufs=2)
                    nc.vector.tensor_scalar(out=lnfk[:], in0=t1[:],
                                            scalar1=fv[:, 0, k:k + 1], scalar2=fv[:, 1, k:k + 1],
                                            op0=ALU.mult, op1=ALU.add)
                    nc.sync.dma_start(
                        d_out.ap()[out_slot[l], 1].rearrange("(k p) t -> p k t", p=128)[:, k, :],
                        lnfk[:])
        if NL < L:
            nc.sync.dma_start(d_out.ap()[0, 0].rearrange("(k p) t -> p k t", p=128), x[:])
    nc.finalize()
    return nc


_NC_CACHE = {}


def kernel(**inputs):
    n_layers = int(inputs.pop("_n_layers", L))
    if n_layers not in _NC_CACHE:
        _NC_CACHE[n_layers] = build_nc(n_layers)
    nc = _NC_CACHE[n_layers]
    in_maps = [_prep_core_inputs(c, inputs) for c in range(NCORES)]
    res = None
    for attempt in range(3):
        try:
            res = run_bass_kernel_spmd(nc, in_maps, core_ids=list(range(NCORES)))
            break
        except Exception:
            if attempt == 2:
                raise
    outs = res.results
    ps = []
    for j in range(4):
        views = []
        for v in range(S):
            ob = outs[4 * v]["outbuf"][j]
            loc = ob[0][:, :T].T
            lnf = ob[1][:, :T].T
            views.append(np.concatenate([loc, lnf], -1))
        ps.append(np.stack(views, 0)[None])
    cam_views = []
    for v in range(S):
        x10_t0 = outs[4 * v]["outbuf"][3, 0][:, 0]
        x11_t0 = outs[4 * v]["outbuf"][4, 0][:, 0]
        cam_views.append(np.concatenate([x10_t0, x11_t0], -1))
    cam = np.stack(cam_views, 0)[None]
    out = tuple(p[:, :, 1:, :].astype(np.float32) for p in ps) + (cam.astype(np.float32),)
    return out
